# revision 28
# baseline (speedup 1.0000x reference)
"""Trainium2 Bass kernel for the Capsule routing layer.

Math (see module docstring of the problem):
    R_nor = softmax(R[0], axis=0)                      # over N, per capsule c
    u[b,n,c,j] = sum_k W[0,n,c,j,k] * x[b,n,k]
    s[b,c,j]   = sum_n u[b,n,c,j] * R_nor[n,c]
    ss = sum_j s^2 + EPS ; out = sqrt(ss)/(1+ss) * s   # squash

Distribution: output-parallel over capsules C=32 across 8 cores (4 capsules
per core).  Each core performs the full contraction over (n,k) = 16384 for
its capsule slice, so there is no cross-core reduction (collectives on TRN2
have a ~20us latency floor which would dominate this ~20us kernel).

Per-core device algorithm:
    s[b,(c,j)] = sum_{(n,k)} x[b,n,k] * W[n,c,j,k] * exp(R[n,c]) / Z_c
    Z_c        = sum_n exp(R[n,c])
exp(R) is folded into W with one broadcast vector multiply per chunk, the
(n,k) contraction runs on the PE as 128 accumulating matmuls of
[p=128] x [m=32(b)] x [f=64(c,j)], and Z comes from the same exp(R) tile via
a free-dim reduce + ones-matmul partition reduce.  Softmax scale 1/Z and the
squash run on the tiny [32,64] result.

Host-side work is layout only (transpose/replicate of the raw inputs into
DMA-friendly shards); all arithmetic happens on device.
"""

import os

import numpy as np

P = 128                    # SBUF partitions
B, N, DIN, C, DOUT = 32, 2048, 8, 32, 16
NCORES = 8
CS = C // NCORES           # capsules per core (4)
NPB = P // DIN             # n's per 128-row block (16)
NBLK = (N * DIN) // P      # nk blocks of 128 (128)
BPC = 8                    # blocks per W chunk
CHUNKS = NBLK // BPC       # 16 chunks of [128, 512]
XT = 4                     # number of x tiles
FW = CS * DOUT             # free width per block (64)
EPS = 1e-7

LAST_EXEC_TIME_NS = None

_compiled = None


def _build(debug=False):
    from concourse import bacc, mybir, tile

    f32 = mybir.dt.float32
    Exp = mybir.ActivationFunctionType.Exp
    Sqrt = mybir.ActivationFunctionType.Sqrt
    AxX = mybir.AxisListType.X

    nc = bacc.Bacc(
        "TRN2", target_bir_lowering=False, debug=debug, num_devices=NCORES
    )
    wp = nc.dram_tensor("w_prep", [P, NBLK * FW], f32, kind="ExternalInput")
    xp = nc.dram_tensor("x_prep", [P, NBLK * B], f32, kind="ExternalInput")
    rp = nc.dram_tensor("r_rep", [P, NBLK * CS], f32, kind="ExternalInput")
    out = nc.dram_tensor("out", [B, FW], f32, kind="ExternalOutput")

    with tile.TileContext(nc) as tc:
        with (
            tc.tile_pool(name="wpool", bufs=6) as wpool,
            tc.tile_pool(name="wepool", bufs=4) as wepool,
            tc.tile_pool(name="misc", bufs=1) as misc,
            tc.tile_pool(name="ppool", bufs=1, space="PSUM") as ppool,
        ):
            # Warm the ACT function tables (Exp, Sqrt) with dummy no-dep
            # activations so the ~1.3us-per-table HBM load overlaps the DMA
            # head instead of sitting on the critical path.
            # (Sqrt's warm-up is emitted after the real Exp below — the ACT
            # table cache holds one function, so warming Sqrt here would
            # evict the Exp table right before the critical-path Exp.)
            warm = misc.tile([1, 1], f32, tag="warm")
            nc.vector.memset(warm[:], 1.0)
            nc.scalar.activation(warm[:], warm[:], Exp)

            # Warm the PE HAM clock gate (cold PE runs at 1.2 GHz for the
            # first ~3.4us of activity): burn ~3.5us of dummy matmuls while
            # the DMA head is still streaming.
            inv_din = misc.tile([P, 1], f32, tag="inv_din")
            nc.vector.memset(inv_din[:], 1.0 / DIN)
            wrm = misc.tile([P, 128], f32, tag="wrm")
            nc.vector.memset(wrm[:], 0.0)
            wrm_ps = ppool.tile([1, 128], f32, tag="wrm_ps")
            for _ in range(4):
                nc.tensor.matmul(wrm_ps[:], inv_din[:], wrm[:])

            # R (k-replicated on host) leads the ACT ring (its FIFO position
            # guarantees it lands first there), followed by the x1-3
            # descriptor-gens; Exp comes after the D2Ds because the in-order
            # scalar sequencer stalls at a blocked engine op.  x0 leads the
            # sync ring ahead of the W stream.
            r_t = misc.tile([P, NBLK * CS], f32, tag="r")
            nc.scalar.dma_start(out=r_t[:], in_=rp[:])
            xcols = NBLK * B // XT
            x_tiles = [
                misc.tile([P, xcols], f32, tag=f"x{i}", name=f"x{i}")
                for i in range(XT)
            ]
            nc.sync.dma_start(out=x_tiles[0][:], in_=xp[:, 0:xcols])
            # Exp is split: a 32-col head unblocks chunk 0's multiply the
            # moment R lands; the tail covers chunks 1-15.  Both are emitted
            # before the x1-3 D2Ds (whose ring-credit stalls would otherwise
            # block the in-order scalar sequencer ahead of Exp).
            ECH = BPC * CS  # e columns per chunk (32)
            e0_t = misc.tile([P, ECH], f32, tag="e0")
            nc.scalar.activation(e0_t[:], r_t[:, 0:ECH], Exp)
            e1_t = misc.tile([P, (CHUNKS - 1) * ECH], f32, tag="e1")
            nc.scalar.activation(e1_t[:], r_t[:, ECH:], Exp)
            # Warm the Sqrt table now; reading e1_t forces it AFTER the Exps.
            nc.scalar.activation(warm[:], e1_t[0:1, 0:1], Sqrt)
            for i in range(1, XT):
                nc.scalar.dma_start(
                    out=x_tiles[i][:], in_=xp[:, i * xcols : (i + 1) * xcols]
                )

            def e_slice(ch):
                if ch == 0:
                    return e0_t[:, :]
                return e1_t[:, (ch - 1) * ECH : ch * ECH]

            # W stream, split across two DMA paths.  The HWDGE rings hold
            # only ~4 in-flight DMAs and each sequencer issues its D2Ds
            # before its cross-engine go-signals, so keeping the sync ring
            # at <=4 D2Ds (x0 + three W DMAs) avoids credit stalls gating
            # the first multiply; the back half of W streams via the gpsimd
            # SWDGE, whose descriptor generation runs independently on Q7.
            W_SPLITS = [  # (engine idx 0=sync/1=gpsimd, ncols)
                (0, 512), (0, 1024), (0, 2048), (1, 2048), (1, 2560),
            ]
            w_tiles = []  # (tile, first_col, ncols)

            def emit_w_dma(eng, ncols):
                col0 = sum(tn for _, _, tn in w_tiles)
                w_t = wpool.tile([P, ncols], f32, tag="w", name=f"w{len(w_tiles)}")
                engine = nc.sync if eng == 0 else nc.gpsimd
                engine.dma_start(out=w_t[:], in_=wp[:, col0 : col0 + ncols])
                w_tiles.append((w_t, col0, ncols))

            for eng, ncols in W_SPLITS:
                emit_w_dma(eng, ncols)

            def w_slice(ch):
                c0 = ch * BPC * FW
                for t, t0, tn in w_tiles:
                    if t0 <= c0 and c0 + BPC * FW <= t0 + tn:
                        return t[:, c0 - t0 : c0 - t0 + BPC * FW]
                raise AssertionError(ch)

            # Z_c = sum_n exp(R[n,c]).  Each n appears DIN times across the
            # partition rows, so reduce blocks on the free axis, then a
            # (1/DIN)-ones matmul reduces partitions.  Emitted mid-loop so
            # the scheduler doesn't prioritize it over the W multiplies.
            def emit_z_path():
                en_a = misc.tile([P, CS], f32, tag="en_a")
                nc.vector.reduce_sum(
                    out=en_a[:],
                    in_=e0_t[:].rearrange("p (blk c) -> p c blk", blk=BPC, c=CS),
                    axis=AxX,
                )
                en_b = misc.tile([P, CS], f32, tag="en_b")
                nc.vector.reduce_sum(
                    out=en_b[:],
                    in_=e1_t[:].rearrange(
                        "p (blk c) -> p c blk", blk=(CHUNKS - 1) * BPC, c=CS
                    ),
                    axis=AxX,
                )
                en_sum = misc.tile([P, CS], f32, tag="en_sum")
                nc.vector.tensor_add(en_sum[:], en_a[:], en_b[:])
                z_ps = ppool.tile([1, CS], f32, tag="z")
                nc.tensor.matmul(z_ps[:], inv_din[:], en_sum[:])
                invz = misc.tile([1, CS], f32, tag="invz")
                nc.vector.reciprocal(invz[:], z_ps[:])
                # Broadcast 1/Z to all 32 batch partitions via rank-1 matmul.
                ones_b = misc.tile([1, B], f32, tag="ones_b")
                nc.vector.memset(ones_b[:], 1.0)
                bc_ps = ppool.tile([B, CS], f32, tag="bc")
                nc.tensor.matmul(bc_ps[:], ones_b[:], invz[:])
                bc_sb = misc.tile([B, CS], f32, tag="bc_sb")
                nc.scalar.copy(bc_sb[:], bc_ps[:])
                return bc_sb

            # Main contraction: per chunk of 8 nk-blocks, fold exp(R[n,c])
            # into W (broadcast over j; k-rows already replicated) and run 8
            # accumulating matmuls into the [32, 64] PSUM tile.
            s_ps = ppool.tile([B, FW], f32, tag="s")
            for ch in range(CHUNKS):
                if ch == 10:
                    bc_sb = emit_z_path()
                we_t = wepool.tile([P, BPC * FW], f32, tag="we")
                e_view = (
                    e_slice(ch)
                    .rearrange("p (blk c) -> p blk c", blk=BPC, c=CS)
                    .unsqueeze(3)
                    .broadcast_to([P, BPC, CS, DOUT])
                )
                nc.vector.tensor_mul(
                    we_t[:].rearrange("p (blk c j) -> p blk c j", blk=BPC, c=CS, j=DOUT),
                    w_slice(ch).rearrange("p (blk c j) -> p blk c j", blk=BPC, c=CS, j=DOUT),
                    e_view,
                )
                for bi in range(BPC):
                    blk = ch * BPC + bi
                    xt_i, xcol = divmod(blk, NBLK // XT)
                    nc.tensor.matmul(
                        s_ps[:],
                        x_tiles[xt_i][:, xcol * B : (xcol + 1) * B],
                        we_t[:, bi * FW : (bi + 1) * FW],
                        start=(blk == 0),
                        stop=(blk == NBLK - 1),
                    )

            # s = s_unnorm / Z ; squash: out = sqrt(ss)/(1+ss) * s
            sn = misc.tile([B, FW], f32, tag="sn")
            nc.vector.tensor_mul(
                sn[:].rearrange("p (c j) -> p c j", c=CS, j=DOUT),
                s_ps[:].rearrange("p (c j) -> p c j", c=CS, j=DOUT),
                bc_sb[:].unsqueeze(2).broadcast_to([B, CS, DOUT]),
            )
            sq = misc.tile([B, FW], f32, tag="sq")
            nc.vector.tensor_mul(sq[:], sn[:], sn[:])
            ss = misc.tile([B, CS], f32, tag="ss")
            nc.vector.reduce_sum(
                out=ss[:],
                in_=sq[:].rearrange("p (c j) -> p c j", c=CS, j=DOUT),
                axis=AxX,
            )
            eps_t = misc.tile([B, 1], f32, tag="eps")
            nc.vector.memset(eps_t[:], EPS)
            sqrt_ss = misc.tile([B, CS], f32, tag="sqrt_ss")
            nc.scalar.activation(sqrt_ss[:], ss[:], Sqrt, bias=eps_t[:])
            den = misc.tile([B, CS], f32, tag="den")
            nc.vector.tensor_scalar_add(den[:], ss[:], 1.0 + EPS)
            rden = misc.tile([B, CS], f32, tag="rden")
            nc.vector.reciprocal(rden[:], den[:])
            scl = misc.tile([B, CS], f32, tag="scl")
            nc.vector.tensor_mul(scl[:], sqrt_ss[:], rden[:])
            o_t = misc.tile([B, FW], f32, tag="o")
            nc.vector.tensor_mul(
                o_t[:].rearrange("p (c j) -> p c j", c=CS, j=DOUT),
                sn[:].rearrange("p (c j) -> p c j", c=CS, j=DOUT),
                scl[:].unsqueeze(2).broadcast_to([B, CS, DOUT]),
            )
            nc.sync.dma_start(out=out[:], in_=o_t[:])

    nc.compile()
    return nc


def _prep(x, W, R):
    """Layout-only host prep: shard + transpose into DMA-friendly tiles.

    Row index everywhere: p = n_in_blk * DIN + k.
    """
    x = np.ascontiguousarray(x, dtype=np.float32)
    W = np.ascontiguousarray(W, dtype=np.float32)
    R = np.ascontiguousarray(R, dtype=np.float32)

    # x_prep[p, blk*B + b] = x[b, n(blk, p), k(p)]   (shared by all cores)
    x_prep = np.ascontiguousarray(
        x.reshape(B, NBLK, NPB, DIN).transpose(2, 3, 1, 0).reshape(P, NBLK * B)
    )

    w_maps, r_maps = [], []
    for i in range(NCORES):
        cs = slice(i * CS, (i + 1) * CS)
        # w_prep[p, blk*FW + c*DOUT + j] = W[0, n(blk,p), c, j, k(p)]
        Wc = W[0][:, cs]  # [N, CS, DOUT, DIN]
        w_maps.append(
            np.ascontiguousarray(
                Wc.reshape(NBLK, NPB, CS, DOUT, DIN)
                .transpose(1, 4, 0, 2, 3)
                .reshape(P, NBLK * FW)
            )
        )
        # r_rep[p, blk*CS + c] = R[0, n(blk,p), c]   (replicated over k)
        Rc = R[0][:, cs].reshape(NBLK, NPB, CS).transpose(1, 0, 2)
        r_maps.append(
            np.ascontiguousarray(
                np.broadcast_to(Rc[:, None], (NPB, DIN, NBLK, CS)).reshape(
                    P, NBLK * CS
                )
            )
        )
    return x_prep, w_maps, r_maps


def kernel(**inputs):
    global _compiled, LAST_EXEC_TIME_NS
    x, W, R = inputs["x"], inputs["W"], inputs["R"]
    if _compiled is None:
        _compiled = _build()
    nc = _compiled

    x_prep, w_maps, r_maps = _prep(np.asarray(x), np.asarray(W), np.asarray(R))
    in_maps = [
        {"w_prep": w_maps[i], "x_prep": x_prep, "r_rep": r_maps[i]}
        for i in range(NCORES)
    ]

    from concourse.bass_utils import run_bass_kernel_spmd

    trace = bool(os.environ.get("BASS_KERNEL_TRACE"))
    res = run_bass_kernel_spmd(nc, in_maps, list(range(NCORES)), trace=trace)
    LAST_EXEC_TIME_NS = res.exec_time_ns

    outs = [res.results[i]["out"].reshape(B, CS, DOUT) for i in range(NCORES)]
    return np.ascontiguousarray(np.concatenate(outs, axis=1))


# revision 29
# speedup vs baseline: 1.0636x; 1.0636x over previous
"""Trainium2 Bass kernel for the Capsule routing layer.

Math (see module docstring of the problem):
    R_nor = softmax(R[0], axis=0)                      # over N, per capsule c
    u[b,n,c,j] = sum_k W[0,n,c,j,k] * x[b,n,k]
    s[b,c,j]   = sum_n u[b,n,c,j] * R_nor[n,c]
    ss = sum_j s^2 + EPS ; out = sqrt(ss)/(1+ss) * s   # squash

Distribution: output-parallel over capsules C=32 across 8 cores (4 capsules
per core).  Each core performs the full contraction over (n,k) = 16384 for
its capsule slice, so there is no cross-core reduction (collectives on TRN2
have a ~20us latency floor which would dominate this ~20us kernel).

Per-core device algorithm:
    s[b,(c,j)] = sum_{(n,k)} x[b,n,k] * W[n,c,j,k] * exp(R[n,c]) / Z_c
    Z_c        = sum_n exp(R[n,c])
exp(R) is folded into W with one broadcast vector multiply per chunk, the
(n,k) contraction runs on the PE as 128 accumulating matmuls of
[p=128] x [m=32(b)] x [f=64(c,j)], and Z comes from the same exp(R) tile via
a free-dim reduce + ones-matmul partition reduce.  Softmax scale 1/Z and the
squash run on the tiny [32,64] result.

Host-side work is layout only (transpose/replicate of the raw inputs into
DMA-friendly shards); all arithmetic happens on device.
"""

import os

import numpy as np

P = 128                    # SBUF partitions
B, N, DIN, C, DOUT = 32, 2048, 8, 32, 16
NCORES = 8
CS = C // NCORES           # capsules per core (4)
NPB = P // DIN             # n's per 128-row block (16)
NBLK = (N * DIN) // P      # nk blocks of 128 (128)
BPC = 8                    # blocks per W chunk
CHUNKS = NBLK // BPC       # 16 chunks of [128, 512]
XT = 4                     # number of x tiles
FW = CS * DOUT             # free width per block (64)
EPS = 1e-7

LAST_EXEC_TIME_NS = None

_compiled = None


def _build(debug=False):
    from concourse import bacc, mybir, tile

    f32 = mybir.dt.float32
    Exp = mybir.ActivationFunctionType.Exp
    Sqrt = mybir.ActivationFunctionType.Sqrt
    AxX = mybir.AxisListType.X

    nc = bacc.Bacc(
        "TRN2", target_bir_lowering=False, debug=debug, num_devices=NCORES
    )
    wp = nc.dram_tensor("w_prep", [P, NBLK * FW], f32, kind="ExternalInput")
    xp = nc.dram_tensor("x_prep", [P, NBLK * B], f32, kind="ExternalInput")
    rp = nc.dram_tensor("r_rep", [P, NBLK * CS], f32, kind="ExternalInput")
    out = nc.dram_tensor("out", [B, FW], f32, kind="ExternalOutput")

    with tile.TileContext(nc) as tc:
        with (
            tc.tile_pool(name="wpool", bufs=6) as wpool,
            tc.tile_pool(name="wepool", bufs=4) as wepool,
            tc.tile_pool(name="misc", bufs=1) as misc,
            tc.tile_pool(name="ppool", bufs=1, space="PSUM") as ppool,
        ):
            # Warm the ACT function tables (Exp, Sqrt) with dummy no-dep
            # activations so the ~1.3us-per-table HBM load overlaps the DMA
            # head instead of sitting on the critical path.
            # (Sqrt's warm-up is emitted after the real Exp below — the ACT
            # table cache holds one function, so warming Sqrt here would
            # evict the Exp table right before the critical-path Exp.)
            warm = misc.tile([1, 1], f32, tag="warm")
            nc.vector.memset(warm[:], 1.0)
            nc.scalar.activation(warm[:], warm[:], Exp)

            # Warm the PE HAM clock gate (cold PE runs at 1.2 GHz for the
            # first ~3.4us of activity): burn ~3.5us of dummy matmuls while
            # the DMA head is still streaming.
            inv_din = misc.tile([P, 1], f32, tag="inv_din")
            nc.vector.memset(inv_din[:], 1.0 / DIN)
            wrm = misc.tile([P, 128], f32, tag="wrm")
            nc.vector.memset(wrm[:], 0.0)
            wrm_ps = ppool.tile([1, 128], f32, tag="wrm_ps")
            for _ in range(4):
                nc.tensor.matmul(wrm_ps[:], inv_din[:], wrm[:])

            # R (k-replicated on host) leads the ACT ring (its FIFO position
            # guarantees it lands first there), followed by the x1-3
            # descriptor-gens; Exp comes after the D2Ds because the in-order
            # scalar sequencer stalls at a blocked engine op.  x0 leads the
            # sync ring ahead of the W stream.
            r_t = misc.tile([P, NBLK * CS], f32, tag="r")
            nc.scalar.dma_start(out=r_t[:], in_=rp[:])
            xcols = NBLK * B // XT
            x_tiles = [
                misc.tile([P, xcols], f32, tag=f"x{i}", name=f"x{i}")
                for i in range(XT)
            ]
            nc.sync.dma_start(out=x_tiles[0][:], in_=xp[:, 0:xcols])
            # Exp is split: a 32-col head unblocks chunk 0's multiply the
            # moment R lands; the tail covers chunks 1-15.  Both are emitted
            # before the x1-3 D2Ds (whose ring-credit stalls would otherwise
            # block the in-order scalar sequencer ahead of Exp).
            ECH = BPC * CS  # e columns per chunk (32)
            e0_t = misc.tile([P, ECH], f32, tag="e0")
            nc.scalar.activation(e0_t[:], r_t[:, 0:ECH], Exp)
            e1_t = misc.tile([P, (CHUNKS - 1) * ECH], f32, tag="e1")
            nc.scalar.activation(e1_t[:], r_t[:, ECH:], Exp)
            # Warm the Sqrt table now; reading e1_t forces it AFTER the Exps.
            nc.scalar.activation(warm[:], e1_t[0:1, 0:1], Sqrt)
            for i in range(1, XT):
                nc.scalar.dma_start(
                    out=x_tiles[i][:], in_=xp[:, i * xcols : (i + 1) * xcols]
                )

            def e_slice(ch):
                if ch == 0:
                    return e0_t[:, :]
                return e1_t[:, (ch - 1) * ECH : ch * ECH]

            # W stream on the sync ring as one small lead DMA + growing
            # tails.  Consumers' go-signals are relayed by the sync
            # sequencer only after its LAST D2D issues, and the ring holds
            # ~4 in-flight DMAs, so 5 D2Ds total (x0 + 4 W) keeps the last
            # issue at ~10us while arrivals stay ahead of the PE's ~240GB/s
            # consumption.
            W_SPLITS = [512, 1024, 2048, 4608]  # cols, sum = 8192
            w_tiles = []  # (tile, first_col, ncols)

            def emit_w_dma(ncols):
                col0 = sum(tn for _, _, tn in w_tiles)
                w_t = wpool.tile([P, ncols], f32, tag="w", name=f"w{len(w_tiles)}")
                nc.sync.dma_start(out=w_t[:], in_=wp[:, col0 : col0 + ncols])
                w_tiles.append((w_t, col0, ncols))

            for ncols in W_SPLITS:
                emit_w_dma(ncols)

            def w_slice(ch):
                c0 = ch * BPC * FW
                for t, t0, tn in w_tiles:
                    if t0 <= c0 and c0 + BPC * FW <= t0 + tn:
                        return t[:, c0 - t0 : c0 - t0 + BPC * FW]
                raise AssertionError(ch)

            # Z_c = sum_n exp(R[n,c]).  Each n appears DIN times across the
            # partition rows, so reduce blocks on the free axis, then a
            # (1/DIN)-ones matmul reduces partitions.  Emitted mid-loop so
            # the scheduler doesn't prioritize it over the W multiplies.
            def emit_z_path():
                en_a = misc.tile([P, CS], f32, tag="en_a")
                nc.vector.reduce_sum(
                    out=en_a[:],
                    in_=e0_t[:].rearrange("p (blk c) -> p c blk", blk=BPC, c=CS),
                    axis=AxX,
                )
                en_b = misc.tile([P, CS], f32, tag="en_b")
                nc.vector.reduce_sum(
                    out=en_b[:],
                    in_=e1_t[:].rearrange(
                        "p (blk c) -> p c blk", blk=(CHUNKS - 1) * BPC, c=CS
                    ),
                    axis=AxX,
                )
                en_sum = misc.tile([P, CS], f32, tag="en_sum")
                nc.vector.tensor_add(en_sum[:], en_a[:], en_b[:])
                z_ps = ppool.tile([1, CS], f32, tag="z")
                nc.tensor.matmul(z_ps[:], inv_din[:], en_sum[:])
                invz = misc.tile([1, CS], f32, tag="invz")
                nc.vector.reciprocal(invz[:], z_ps[:])
                # Broadcast 1/Z to all 32 batch partitions via rank-1 matmul.
                ones_b = misc.tile([1, B], f32, tag="ones_b")
                nc.vector.memset(ones_b[:], 1.0)
                bc_ps = ppool.tile([B, CS], f32, tag="bc")
                nc.tensor.matmul(bc_ps[:], ones_b[:], invz[:])
                bc_sb = misc.tile([B, CS], f32, tag="bc_sb")
                nc.scalar.copy(bc_sb[:], bc_ps[:])
                return bc_sb

            # Main contraction: per chunk of 8 nk-blocks, fold exp(R[n,c])
            # into W (broadcast over j; k-rows already replicated) and run 8
            # accumulating matmuls into the [32, 64] PSUM tile.
            s_ps = ppool.tile([B, FW], f32, tag="s")
            for ch in range(CHUNKS):
                if ch == 10:
                    bc_sb = emit_z_path()
                we_t = wepool.tile([P, BPC * FW], f32, tag="we")
                e_view = (
                    e_slice(ch)
                    .rearrange("p (blk c) -> p blk c", blk=BPC, c=CS)
                    .unsqueeze(3)
                    .broadcast_to([P, BPC, CS, DOUT])
                )
                nc.vector.tensor_mul(
                    we_t[:].rearrange("p (blk c j) -> p blk c j", blk=BPC, c=CS, j=DOUT),
                    w_slice(ch).rearrange("p (blk c j) -> p blk c j", blk=BPC, c=CS, j=DOUT),
                    e_view,
                )
                for bi in range(BPC):
                    blk = ch * BPC + bi
                    xt_i, xcol = divmod(blk, NBLK // XT)
                    nc.tensor.matmul(
                        s_ps[:],
                        x_tiles[xt_i][:, xcol * B : (xcol + 1) * B],
                        we_t[:, bi * FW : (bi + 1) * FW],
                        start=(blk == 0),
                        stop=(blk == NBLK - 1),
                    )

            # s = s_unnorm / Z ; squash: out = sqrt(ss)/(1+ss) * s
            sn = misc.tile([B, FW], f32, tag="sn")
            nc.vector.tensor_mul(
                sn[:].rearrange("p (c j) -> p c j", c=CS, j=DOUT),
                s_ps[:].rearrange("p (c j) -> p c j", c=CS, j=DOUT),
                bc_sb[:].unsqueeze(2).broadcast_to([B, CS, DOUT]),
            )
            sq = misc.tile([B, FW], f32, tag="sq")
            nc.vector.tensor_mul(sq[:], sn[:], sn[:])
            ss = misc.tile([B, CS], f32, tag="ss")
            nc.vector.reduce_sum(
                out=ss[:],
                in_=sq[:].rearrange("p (c j) -> p c j", c=CS, j=DOUT),
                axis=AxX,
            )
            eps_t = misc.tile([B, 1], f32, tag="eps")
            nc.vector.memset(eps_t[:], EPS)
            sqrt_ss = misc.tile([B, CS], f32, tag="sqrt_ss")
            nc.scalar.activation(sqrt_ss[:], ss[:], Sqrt, bias=eps_t[:])
            den = misc.tile([B, CS], f32, tag="den")
            nc.vector.tensor_scalar_add(den[:], ss[:], 1.0 + EPS)
            rden = misc.tile([B, CS], f32, tag="rden")
            nc.vector.reciprocal(rden[:], den[:])
            scl = misc.tile([B, CS], f32, tag="scl")
            nc.vector.tensor_mul(scl[:], sqrt_ss[:], rden[:])
            o_t = misc.tile([B, FW], f32, tag="o")
            nc.vector.tensor_mul(
                o_t[:].rearrange("p (c j) -> p c j", c=CS, j=DOUT),
                sn[:].rearrange("p (c j) -> p c j", c=CS, j=DOUT),
                scl[:].unsqueeze(2).broadcast_to([B, CS, DOUT]),
            )
            nc.sync.dma_start(out=out[:], in_=o_t[:])

    nc.compile()
    return nc


def _prep(x, W, R):
    """Layout-only host prep: shard + transpose into DMA-friendly tiles.

    Row index everywhere: p = n_in_blk * DIN + k.
    """
    x = np.ascontiguousarray(x, dtype=np.float32)
    W = np.ascontiguousarray(W, dtype=np.float32)
    R = np.ascontiguousarray(R, dtype=np.float32)

    # x_prep[p, blk*B + b] = x[b, n(blk, p), k(p)]   (shared by all cores)
    x_prep = np.ascontiguousarray(
        x.reshape(B, NBLK, NPB, DIN).transpose(2, 3, 1, 0).reshape(P, NBLK * B)
    )

    w_maps, r_maps = [], []
    for i in range(NCORES):
        cs = slice(i * CS, (i + 1) * CS)
        # w_prep[p, blk*FW + c*DOUT + j] = W[0, n(blk,p), c, j, k(p)]
        Wc = W[0][:, cs]  # [N, CS, DOUT, DIN]
        w_maps.append(
            np.ascontiguousarray(
                Wc.reshape(NBLK, NPB, CS, DOUT, DIN)
                .transpose(1, 4, 0, 2, 3)
                .reshape(P, NBLK * FW)
            )
        )
        # r_rep[p, blk*CS + c] = R[0, n(blk,p), c]   (replicated over k)
        Rc = R[0][:, cs].reshape(NBLK, NPB, CS).transpose(1, 0, 2)
        r_maps.append(
            np.ascontiguousarray(
                np.broadcast_to(Rc[:, None], (NPB, DIN, NBLK, CS)).reshape(
                    P, NBLK * CS
                )
            )
        )
    return x_prep, w_maps, r_maps


def kernel(**inputs):
    global _compiled, LAST_EXEC_TIME_NS
    x, W, R = inputs["x"], inputs["W"], inputs["R"]
    if _compiled is None:
        _compiled = _build()
    nc = _compiled

    x_prep, w_maps, r_maps = _prep(np.asarray(x), np.asarray(W), np.asarray(R))
    in_maps = [
        {"w_prep": w_maps[i], "x_prep": x_prep, "r_rep": r_maps[i]}
        for i in range(NCORES)
    ]

    from concourse.bass_utils import run_bass_kernel_spmd

    trace = bool(os.environ.get("BASS_KERNEL_TRACE"))
    res = run_bass_kernel_spmd(nc, in_maps, list(range(NCORES)), trace=trace)
    LAST_EXEC_TIME_NS = res.exec_time_ns

    outs = [res.results[i]["out"].reshape(B, CS, DOUT) for i in range(NCORES)]
    return np.ascontiguousarray(np.concatenate(outs, axis=1))


# revision 33
# speedup vs baseline: 1.2624x; 1.1869x over previous
"""Trainium2 Bass kernel for the Capsule routing layer.

Math (see module docstring of the problem):
    R_nor = softmax(R[0], axis=0)                      # over N, per capsule c
    u[b,n,c,j] = sum_k W[0,n,c,j,k] * x[b,n,k]
    s[b,c,j]   = sum_n u[b,n,c,j] * R_nor[n,c]
    ss = sum_j s^2 + EPS ; out = sqrt(ss)/(1+ss) * s   # squash

Distribution: output-parallel over capsules C=32 across 8 cores (4 capsules
per core).  Each core performs the full contraction over (n,k) = 16384 for
its capsule slice, so there is no cross-core reduction (collectives on TRN2
have a ~20us latency floor which would dominate this ~20us kernel).

Per-core device algorithm:
    s[b,(c,j)] = sum_{(n,k)} x[b,n,k] * W[n,c,j,k] * exp(R[n,c]) / Z_c
    Z_c        = sum_n exp(R[n,c])
exp(R) is folded into W with one broadcast vector multiply per chunk, the
(n,k) contraction runs on the PE as 128 accumulating matmuls of
[p=128] x [m=32(b)] x [f=64(c,j)], and Z comes from the same exp(R) tile via
a free-dim reduce + ones-matmul partition reduce.  Softmax scale 1/Z and the
squash run on the tiny [32,64] result.

Host-side work is layout only (transpose/replicate of the raw inputs into
DMA-friendly shards); all arithmetic happens on device.
"""

import os

import numpy as np

P = 128                    # SBUF partitions
B, N, DIN, C, DOUT = 32, 2048, 8, 32, 16
NCORES = 8
CS = C // NCORES           # capsules per core (4)
NPB = P // DIN             # n's per 128-row block (16)
NBLK = (N * DIN) // P      # nk blocks of 128 (128)
BPC = 8                    # blocks per W chunk
CHUNKS = NBLK // BPC       # 16 chunks of [128, 512]
XT = 4                     # number of x tiles
FW = CS * DOUT             # free width per block (64)
EPS = 1e-7

LAST_EXEC_TIME_NS = None

_compiled = None


def _build(debug=False):
    from concourse import bacc, mybir, tile

    f32 = mybir.dt.float32
    Exp = mybir.ActivationFunctionType.Exp
    Sqrt = mybir.ActivationFunctionType.Sqrt
    AxX = mybir.AxisListType.X

    nc = bacc.Bacc(
        "TRN2", target_bir_lowering=False, debug=debug, num_devices=NCORES
    )
    wp = nc.dram_tensor("w_prep", [P, NBLK * FW], f32, kind="ExternalInput")
    xp = nc.dram_tensor("x_prep", [P, NBLK * B], f32, kind="ExternalInput")
    rp = nc.dram_tensor("r_rep", [P, NBLK * CS], f32, kind="ExternalInput")
    out = nc.dram_tensor("out", [B, FW], f32, kind="ExternalOutput")

    with tile.TileContext(nc) as tc:
        with (
            tc.tile_pool(name="wpool", bufs=6) as wpool,
            tc.tile_pool(name="wepool", bufs=4) as wepool,
            tc.tile_pool(name="misc", bufs=1) as misc,
            tc.tile_pool(name="ppool", bufs=1, space="PSUM") as ppool,
        ):
            # Warm the ACT function tables (Exp, Sqrt) with dummy no-dep
            # activations so the ~1.3us-per-table HBM load overlaps the DMA
            # head instead of sitting on the critical path.
            # (Sqrt's warm-up is emitted after the real Exp below — the ACT
            # table cache holds one function, so warming Sqrt here would
            # evict the Exp table right before the critical-path Exp.)
            warm = misc.tile([1, 1], f32, tag="warm")
            nc.vector.memset(warm[:], 1.0)
            nc.scalar.activation(warm[:], warm[:], Exp)

            # Warm the PE HAM clock gate (cold PE runs at 1.2 GHz for the
            # first ~3.4us of activity): burn ~3.5us of dummy matmuls while
            # the DMA head is still streaming.
            inv_din = misc.tile([P, 1], f32, tag="inv_din")
            nc.vector.memset(inv_din[:], 1.0 / DIN)
            wrm = misc.tile([P, 128], f32, tag="wrm")
            nc.vector.memset(wrm[:], 0.0)
            wrm_ps = ppool.tile([1, 128], f32, tag="wrm_ps")
            for _ in range(4):
                nc.tensor.matmul(wrm_ps[:], inv_din[:], wrm[:])

            # R (k-replicated on host) leads the ACT ring (its FIFO position
            # guarantees it lands first there), followed by the x1-3
            # descriptor-gens; Exp comes after the D2Ds because the in-order
            # scalar sequencer stalls at a blocked engine op.  x0 leads the
            # sync ring ahead of the W stream.
            w_tiles = []  # (tile, first_col, ncols)

            def emit_w_dma(engine, ncols):
                col0 = sum(tn for _, _, tn in w_tiles)
                w_t = wpool.tile([P, ncols], f32, tag="w", name=f"w{len(w_tiles)}")
                engine.dma_start(out=w_t[:], in_=wp[:, col0 : col0 + ncols])
                w_tiles.append((w_t, col0, ncols))

            r_t = misc.tile([P, NBLK * CS], f32, tag="r")
            nc.scalar.dma_start(out=r_t[:], in_=rp[:])
            xcols = NBLK * B // XT
            x_tiles = [
                misc.tile([P, xcols], f32, tag=f"x{i}", name=f"x{i}")
                for i in range(XT)
            ]
            nc.sync.dma_start(out=x_tiles[0][:], in_=xp[:, 0:xcols])
            for ncols in (512, 512, 1024, 2048):
                emit_w_dma(nc.sync, ncols)
            # Exp is split: a 32-col head unblocks chunk 0's multiply the
            # moment R lands; the tail covers chunks 1-15.  Both are emitted
            # before the x1-3 D2Ds (whose ring-credit stalls would otherwise
            # block the in-order scalar sequencer ahead of Exp).
            ECH = BPC * CS  # e columns per chunk (32)
            e0_t = misc.tile([P, ECH], f32, tag="e0")
            nc.scalar.activation(e0_t[:], r_t[:, 0:ECH], Exp)
            e1_t = misc.tile([P, (CHUNKS - 1) * ECH], f32, tag="e1")
            nc.scalar.activation(e1_t[:], r_t[:, ECH:], Exp)
            # Warm the Sqrt table now; reading e1_t forces it AFTER the Exps.
            nc.scalar.activation(warm[:], e1_t[0:1, 0:1], Sqrt)
            for i in range(1, XT):
                nc.scalar.dma_start(
                    out=x_tiles[i][:], in_=xp[:, i * xcols : (i + 1) * xcols]
                )
            for ncols in (2048, 2048):
                emit_w_dma(nc.scalar, ncols)

            def e_slice(ch):
                if ch == 0:
                    return e0_t[:, :]
                return e1_t[:, (ch - 1) * ECH : ch * ECH]

            def w_slice(ch):
                c0 = ch * BPC * FW
                for t, t0, tn in w_tiles:
                    if t0 <= c0 and c0 + BPC * FW <= t0 + tn:
                        return t[:, c0 - t0 : c0 - t0 + BPC * FW]
                raise AssertionError(ch)

            # Z_c = sum_n exp(R[n,c]).  Each n appears DIN times across the
            # partition rows, so reduce blocks on the free axis, then a
            # (1/DIN)-ones matmul reduces partitions.  Emitted mid-loop so
            # the scheduler doesn't prioritize it over the W multiplies.
            def emit_z_path():
                en_a = misc.tile([P, CS], f32, tag="en_a")
                nc.vector.reduce_sum(
                    out=en_a[:],
                    in_=e0_t[:].rearrange("p (blk c) -> p c blk", blk=BPC, c=CS),
                    axis=AxX,
                )
                en_b = misc.tile([P, CS], f32, tag="en_b")
                nc.vector.reduce_sum(
                    out=en_b[:],
                    in_=e1_t[:].rearrange(
                        "p (blk c) -> p c blk", blk=(CHUNKS - 1) * BPC, c=CS
                    ),
                    axis=AxX,
                )
                en_sum = misc.tile([P, CS], f32, tag="en_sum")
                nc.vector.tensor_add(en_sum[:], en_a[:], en_b[:])
                z_ps = ppool.tile([1, CS], f32, tag="z")
                nc.tensor.matmul(z_ps[:], inv_din[:], en_sum[:])
                invz = misc.tile([1, CS], f32, tag="invz")
                nc.vector.reciprocal(invz[:], z_ps[:])
                # Broadcast 1/Z to all 32 batch partitions via rank-1 matmul.
                ones_b = misc.tile([1, B], f32, tag="ones_b")
                nc.vector.memset(ones_b[:], 1.0)
                bc_ps = ppool.tile([B, CS], f32, tag="bc")
                nc.tensor.matmul(bc_ps[:], ones_b[:], invz[:])
                bc_sb = misc.tile([B, CS], f32, tag="bc_sb")
                nc.scalar.copy(bc_sb[:], bc_ps[:])
                return bc_sb

            # Main contraction: per chunk of 8 nk-blocks, fold exp(R[n,c])
            # into W (broadcast over j; k-rows already replicated) and run 8
            # accumulating matmuls into the [32, 64] PSUM tile.
            s_ps = ppool.tile([B, FW], f32, tag="s")
            for ch in range(CHUNKS):
                if ch == 10:
                    bc_sb = emit_z_path()
                we_t = wepool.tile([P, BPC * FW], f32, tag="we")
                e_view = (
                    e_slice(ch)
                    .rearrange("p (blk c) -> p blk c", blk=BPC, c=CS)
                    .unsqueeze(3)
                    .broadcast_to([P, BPC, CS, DOUT])
                )
                nc.vector.tensor_mul(
                    we_t[:].rearrange("p (blk c j) -> p blk c j", blk=BPC, c=CS, j=DOUT),
                    w_slice(ch).rearrange("p (blk c j) -> p blk c j", blk=BPC, c=CS, j=DOUT),
                    e_view,
                )
                for bi in range(BPC):
                    blk = ch * BPC + bi
                    xt_i, xcol = divmod(blk, NBLK // XT)
                    nc.tensor.matmul(
                        s_ps[:],
                        x_tiles[xt_i][:, xcol * B : (xcol + 1) * B],
                        we_t[:, bi * FW : (bi + 1) * FW],
                        start=(blk == 0),
                        stop=(blk == NBLK - 1),
                    )

            # s = s_unnorm / Z ; squash: out = sqrt(ss)/(1+ss) * s
            sn = misc.tile([B, FW], f32, tag="sn")
            nc.vector.tensor_mul(
                sn[:].rearrange("p (c j) -> p c j", c=CS, j=DOUT),
                s_ps[:].rearrange("p (c j) -> p c j", c=CS, j=DOUT),
                bc_sb[:].unsqueeze(2).broadcast_to([B, CS, DOUT]),
            )
            sq = misc.tile([B, FW], f32, tag="sq")
            nc.vector.tensor_mul(sq[:], sn[:], sn[:])
            ss = misc.tile([B, CS], f32, tag="ss")
            nc.vector.reduce_sum(
                out=ss[:],
                in_=sq[:].rearrange("p (c j) -> p c j", c=CS, j=DOUT),
                axis=AxX,
            )
            eps_t = misc.tile([B, 1], f32, tag="eps")
            nc.vector.memset(eps_t[:], EPS)
            sqrt_ss = misc.tile([B, CS], f32, tag="sqrt_ss")
            nc.scalar.activation(sqrt_ss[:], ss[:], Sqrt, bias=eps_t[:])
            den = misc.tile([B, CS], f32, tag="den")
            nc.vector.tensor_scalar_add(den[:], ss[:], 1.0 + EPS)
            rden = misc.tile([B, CS], f32, tag="rden")
            nc.vector.reciprocal(rden[:], den[:])
            scl = misc.tile([B, CS], f32, tag="scl")
            nc.vector.tensor_mul(scl[:], sqrt_ss[:], rden[:])
            o_t = misc.tile([B, FW], f32, tag="o")
            nc.vector.tensor_mul(
                o_t[:].rearrange("p (c j) -> p c j", c=CS, j=DOUT),
                sn[:].rearrange("p (c j) -> p c j", c=CS, j=DOUT),
                scl[:].unsqueeze(2).broadcast_to([B, CS, DOUT]),
            )
            nc.sync.dma_start(out=out[:], in_=o_t[:])

    nc.compile()
    return nc


def _prep(x, W, R):
    """Layout-only host prep: shard + transpose into DMA-friendly tiles.

    Row index everywhere: p = n_in_blk * DIN + k.
    """
    x = np.ascontiguousarray(x, dtype=np.float32)
    W = np.ascontiguousarray(W, dtype=np.float32)
    R = np.ascontiguousarray(R, dtype=np.float32)

    # x_prep[p, blk*B + b] = x[b, n(blk, p), k(p)]   (shared by all cores)
    x_prep = np.ascontiguousarray(
        x.reshape(B, NBLK, NPB, DIN).transpose(2, 3, 1, 0).reshape(P, NBLK * B)
    )

    w_maps, r_maps = [], []
    for i in range(NCORES):
        cs = slice(i * CS, (i + 1) * CS)
        # w_prep[p, blk*FW + c*DOUT + j] = W[0, n(blk,p), c, j, k(p)]
        Wc = W[0][:, cs]  # [N, CS, DOUT, DIN]
        w_maps.append(
            np.ascontiguousarray(
                Wc.reshape(NBLK, NPB, CS, DOUT, DIN)
                .transpose(1, 4, 0, 2, 3)
                .reshape(P, NBLK * FW)
            )
        )
        # r_rep[p, blk*CS + c] = R[0, n(blk,p), c]   (replicated over k)
        Rc = R[0][:, cs].reshape(NBLK, NPB, CS).transpose(1, 0, 2)
        r_maps.append(
            np.ascontiguousarray(
                np.broadcast_to(Rc[:, None], (NPB, DIN, NBLK, CS)).reshape(
                    P, NBLK * CS
                )
            )
        )
    return x_prep, w_maps, r_maps


def kernel(**inputs):
    global _compiled, LAST_EXEC_TIME_NS
    x, W, R = inputs["x"], inputs["W"], inputs["R"]
    if _compiled is None:
        _compiled = _build()
    nc = _compiled

    x_prep, w_maps, r_maps = _prep(np.asarray(x), np.asarray(W), np.asarray(R))
    in_maps = [
        {"w_prep": w_maps[i], "x_prep": x_prep, "r_rep": r_maps[i]}
        for i in range(NCORES)
    ]

    from concourse.bass_utils import run_bass_kernel_spmd

    trace = bool(os.environ.get("BASS_KERNEL_TRACE"))
    res = run_bass_kernel_spmd(nc, in_maps, list(range(NCORES)), trace=trace)
    LAST_EXEC_TIME_NS = res.exec_time_ns

    outs = [res.results[i]["out"].reshape(B, CS, DOUT) for i in range(NCORES)]
    return np.ascontiguousarray(np.concatenate(outs, axis=1))
